# revision 1
# baseline (speedup 1.0000x reference)
"""Trainium2 Bass kernel for packed varlen causal attention (8 seqs x 1024 tok).

Sharding: data-parallel over sequences -- core i computes sequence i end to end.
Weights are replicated. No collectives.

Device-side math (per core, S=1024 tokens, E=1024, H=16, D=64):
  QT[e,t] = (0.125*Wq)^T-matmul, + 0.125*bq      (scale folded into weights)
  KT[e,t] = Wk^T-matmul
  V [t,e] = Wv^T-matmul, stored head-major with a ones column per head
  per head h, per q-block (512 wide):
    for k-tile (128 rows, causally live only):
      scoresT[k,q] = KT_h tile^T-matmul QT_h      (PSUM, fp32)
      p = exp(scoresT)                            (ScalarE, -> bf16 SBUF)
      causal zero-fill on the diagonal tile       (GpSimd affine_select)
      acc[d+1, q] += [V_h | 1]^T-matmul p         (PSUM accumulate)
    row d of acc = softmax denominator; rows 0..63 = unnormalized (PV)^T
  normalize with one batched reciprocal + broadcast multiply
  outT[e,t] = Wo^T-matmul A^T + (bo + Wo@bv)     (bv folded: softmax rows sum to 1)

Host glue transposes X/W (bf16) on the way in and out^T back on the way out.
"""

import numpy as np
import ml_dtypes

# Problem constants (hardcoded per the harness contract).
NUM_SEQS = 8
SEQ = 1024
EMBED = 1024
HEADS = 16
HEAD_DIM = 64
P = 128
NK = EMBED // P          # 8 contraction tiles
QB = 512                 # q-block width
NQB = SEQ // QB          # 2 q-blocks
HV = HEAD_DIM + 1        # V columns per head incl. ones column

_CACHE = {}


def build_module(reps=1):
    """Build and compile the SPMD Bass module. reps>1 wraps the body in a
    hardware loop (used only for wall-clock timing in test harnesses)."""
    import os
    import concourse.mybir as mybir
    import concourse.tile as tile
    from concourse import bacc
    from contextlib import ExitStack

    # Ablation knobs for perf bisection only; graded path uses the defaults.
    phases = int(os.environ.get("KERNEL_PHASES", "4"))
    no_evict = os.environ.get("KERNEL_NO_EVICT", "0") == "1"

    bf16 = mybir.dt.bfloat16
    f32 = mybir.dt.float32
    EXP = mybir.ActivationFunctionType.Exp

    nc = bacc.Bacc("TRN2", target_bir_lowering=False, debug=False,
                   num_devices=NUM_SEQS, num_swdge_queues=4)

    xt_d = nc.dram_tensor("xt", [EMBED, SEQ], bf16, kind="ExternalInput").ap()
    wq_d = nc.dram_tensor("wqt", [EMBED, EMBED], bf16, kind="ExternalInput").ap()
    wk_d = nc.dram_tensor("wkt", [EMBED, EMBED], bf16, kind="ExternalInput").ap()
    wv_d = nc.dram_tensor("wvt", [EMBED, EMBED], bf16, kind="ExternalInput").ap()
    wo_d = nc.dram_tensor("wot", [EMBED, EMBED], bf16, kind="ExternalInput").ap()
    bq_d = nc.dram_tensor("bqs", [EMBED], f32, kind="ExternalInput").ap()
    bo_d = nc.dram_tensor("boe", [EMBED], f32, kind="ExternalInput").ap()
    ot_d = nc.dram_tensor("ot", [EMBED, SEQ], f32, kind="ExternalOutput").ap()

    xt_v = xt_d.rearrange("(a p) t -> a p t", p=P)
    wq_v = wq_d.rearrange("(a p) e -> a p e", p=P)
    wk_v = wk_d.rearrange("(a p) e -> a p e", p=P)
    wv_v = wv_d.rearrange("(a p) e -> a p e", p=P)
    wo_v = wo_d.rearrange("(a p) e -> a p e", p=P)

    with tile.TileContext(nc) as tc:
        with ExitStack() as ctx:
            const = ctx.enter_context(tc.tile_pool(name="const", bufs=1))
            pp_mm = ctx.enter_context(
                tc.tile_pool(name="pp_mm", bufs=4, space="PSUM"))
            pp_sc = ctx.enter_context(
                tc.tile_pool(name="pp_sc", bufs=4, space="PSUM"))
            pexp = ctx.enter_context(tc.tile_pool(name="pexp", bufs=6))
            pdn = ctx.enter_context(tc.tile_pool(name="pdn", bufs=2))
            prc = ctx.enter_context(tc.tile_pool(name="prc", bufs=2))
            postg = ctx.enter_context(tc.tile_pool(name="postg", bufs=4))
            pdram = ctx.enter_context(
                tc.tile_pool(name="pdram", bufs=2, space="DRAM"))

            def body(_it=None):
                # --- persistent SBUF tensors ------------------------------
                wq = [const.tile([P, EMBED], bf16, tag=f"wq{k}", name=f"wq{k}") for k in range(NK)]
                wk = [const.tile([P, EMBED], bf16, tag=f"wk{k}", name=f"wk{k}") for k in range(NK)]
                wv = [const.tile([P, EMBED], bf16, tag=f"wv{k}", name=f"wv{k}") for k in range(NK)]
                wo = [const.tile([P, EMBED], bf16, tag=f"wo{k}", name=f"wo{k}") for k in range(NK)]
                xt = [const.tile([P, SEQ], bf16, tag=f"xt{k}", name=f"xt{k}") for k in range(NK)]
                qt = [const.tile([P, SEQ], bf16, tag=f"qt{a}", name=f"qt{a}") for a in range(NK)]
                kt = [const.tile([P, SEQ], bf16, tag=f"kt{a}", name=f"kt{a}") for a in range(NK)]
                vv = [const.tile([P, HEADS * HV], bf16, tag=f"vv{m}", name=f"vv{m}")
                      for m in range(NK)]
                at = [const.tile([P, SEQ], bf16, tag=f"at{a}", name=f"at{a}") for a in range(NK)]
                bqs = const.tile([P, NK], f32, tag="bqs")
                boe = const.tile([P, NK], f32, tag="boe")
                # explicit zero bias for Exp: a float bias would lazily
                # allocate a bass-level const tensor outside the tile pools'
                # allocator, which can land under a pool slot.
                zb = const.tile([P, 1], f32, tag="zb")
                nc.vector.memset(zb, 0.0)

                # --- loads ------------------------------------------------
                # one HWDGE queue sustains only ~22 GB/s; round-robin issue
                # over SP + ACT (HWDGE) and POOL (SWDGE) to reach ~170 GB/s.
                dma_engines = [nc.sync, nc.scalar, nc.gpsimd]
                _di = [0]

                def dma(out, in_):
                    dma_engines[_di[0] % len(dma_engines)].dma_start(
                        out=out, in_=in_)
                    _di[0] += 1

                dma(bqs, bq_d.rearrange("(p a) -> p a", a=NK))
                dma(boe, bo_d.rearrange("(p a) -> p a", a=NK))
                for k in range(NK):
                    dma(xt[k], xt_v[k])
                    dma(wv[k], wv_v[k])
                for k in range(NK):
                    dma(wq[k], wq_v[k])
                    dma(wk[k], wk_v[k])
                for k in range(NK):
                    dma(wo[k], wo_v[k])
                for m in range(NK):
                    # ones column per head for the fused denominator
                    nc.vector.memset(
                        vv[m].rearrange("p (h c) -> p h c", c=HV)[:, :, HEAD_DIM:HV],
                        1.0)

                def dummy_out(src):
                    ob = postg.tile([P, QB], f32, name="ob", tag="ob")
                    nc.vector.tensor_copy(out=ob, in_=src)
                    nc.sync.dma_start(out=ot_d[0:P, 0:QB], in_=ob)

                if phases < 2:
                    dummy_out(xt[0][:, 0:QB])
                    return

                # --- projections ------------------------------------------
                # Four interleaved PSUM accumulation chains: back-to-back
                # matmuls into the SAME bank stall the PE ~150ns each
                # (measured); round-robining 4 banks hides it, and each
                # stationary tile feeds 2 moving blocks per LDWEIGHTS.
                # V[t,e]: lhsT = X^T tile [c,t], rhs = Wv^T [c,e]
                for mp in range(NK // 2):
                    ms = [slice((2 * mp + i) * P, (2 * mp + i + 1) * P)
                          for i in range(2)]
                    ps = [pp_mm.tile([P, QB], f32, name="psv", tag="ps")
                          for _ in range(4)]
                    for k in range(NK):
                        se = (k == 0), (k == NK - 1)
                        for i in range(2):
                            for n in range(NQB):
                                nc.tensor.matmul(
                                    ps[2 * i + n], lhsT=xt[k][:, ms[i]],
                                    rhs=wv[k][:, n * QB:(n + 1) * QB],
                                    start=se[0], stop=se[1])
                    if not no_evict:
                        for i in range(2):
                            for n in range(NQB):
                                # scatter heads into the HV-strided layout
                                nc.vector.tensor_copy(
                                    out=vv[2 * mp + i]
                                    [:, n * 8 * HV:(n + 1) * 8 * HV]
                                    .rearrange("p (h c) -> p h c", c=HV)
                                    [:, :, 0:HEAD_DIM],
                                    in_=ps[2 * i + n].rearrange(
                                        "p (h c) -> p h c", c=HEAD_DIM))
                # QT[e,t], KT[e,t]: lhsT = W^T tile [c,e], rhs = X^T [c,t]
                for a in range(NK):
                    es = slice(a * P, (a + 1) * P)
                    psq = [pp_mm.tile([P, QB], f32, name="psq", tag="ps")
                           for _ in range(NQB)]
                    psk = [pp_mm.tile([P, QB], f32, name="psk", tag="ps")
                           for _ in range(NQB)]
                    for k in range(NK):
                        se = (k == 0), (k == NK - 1)
                        for n in range(NQB):
                            nc.tensor.matmul(
                                psq[n], lhsT=wq[k][:, es],
                                rhs=xt[k][:, n * QB:(n + 1) * QB],
                                start=se[0], stop=se[1])
                        for n in range(NQB):
                            nc.tensor.matmul(
                                psk[n], lhsT=wk[k][:, es],
                                rhs=xt[k][:, n * QB:(n + 1) * QB],
                                start=se[0], stop=se[1])
                    if not no_evict:
                        for n in range(NQB):
                            ts = slice(n * QB, (n + 1) * QB)
                            nc.vector.tensor_scalar(
                                out=qt[a][:, ts], in0=psq[n],
                                scalar1=bqs[:, a:a + 1], scalar2=None,
                                op0=mybir.AluOpType.add)
                            nc.vector.tensor_copy(out=kt[a][:, ts],
                                                  in_=psk[n])

                if phases < 3:
                    dummy_out(xt[0][:, 0:QB] if no_evict else qt[0][:, 0:QB])
                    return

                # --- attention --------------------------------------------
                # kb-outer / qb-inner: consecutive matmuls share stationary
                # weights (one KT tile, then one V tile), and the two q-block
                # accumulation chains interleave so PE never waits on exp.
                NKB = SEQ // P
                denp = None
                for h in range(HEADS):
                    a_h = h // 2
                    po = (h % 2) * HEAD_DIM
                    hvs = slice(h * HV, h * HV + HV)
                    acc = [pp_mm.tile([P, QB], f32, name="acc", tag="ps")
                           for qb in range(NQB)]
                    if h % 2 == 0:
                        denp = pdn.tile([2, SEQ], f32, name="denp", tag="denp")
                    dp = denp

                    def evict(qb):
                        qs = slice(qb * QB, (qb + 1) * QB)
                        # engines can only address partitions {0,32,64,96};
                        # bounce the denominator row via SBUF->SBUF DMA.
                        ds = postg.tile([1, QB], f32, name="ds", tag="ds")
                        nc.vector.tensor_copy(out=ds,
                                              in_=acc[qb][HEAD_DIM:HV, :])
                        nc.sync.dma_start(out=dp[h % 2:h % 2 + 1, qs], in_=ds)
                        nc.vector.tensor_copy(
                            out=at[a_h][po:po + HEAD_DIM, qs],
                            in_=acc[qb][0:HEAD_DIM, :])

                    for kb in range(NKB):
                        elig = [qb for qb in range(NQB)
                                if (kb + 1) * P <= (qb + 1) * QB]
                        c0 = {qb: max(0, kb * P - qb * QB) for qb in elig}
                        sc = {}
                        for qb in elig:
                            sc[qb] = pp_sc.tile([P, QB], f32, name="sc")
                            nc.tensor.matmul(
                                sc[qb][:, c0[qb]:QB],
                                lhsT=kt[a_h][po:po + HEAD_DIM,
                                             kb * P:(kb + 1) * P],
                                rhs=qt[a_h][po:po + HEAD_DIM,
                                            qb * QB + c0[qb]:(qb + 1) * QB],
                                start=True, stop=True)
                        pt = {}
                        for qb in elig:
                            pt[qb] = pexp.tile([P, QB], bf16, name="pt")
                            nc.scalar.activation(out=pt[qb][:, c0[qb]:QB],
                                                 in_=sc[qb][:, c0[qb]:QB],
                                                 func=EXP, bias=zb)
                            if kb * P >= qb * QB:
                                # diagonal tile: zero strictly-upper triangle
                                nc.gpsimd.affine_select(
                                    out=pt[qb][:, c0[qb]:c0[qb] + P],
                                    in_=pt[qb][:, c0[qb]:c0[qb] + P],
                                    compare_op=mybir.AluOpType.is_ge,
                                    fill=0.0, base=0,
                                    pattern=[[1, P]], channel_multiplier=-1)
                        for qb in elig:
                            last = kb == (qb + 1) * (QB // P) - 1
                            nc.tensor.matmul(
                                acc[qb][:HV, c0[qb]:QB], lhsT=vv[kb][:, hvs],
                                rhs=pt[qb][:, c0[qb]:QB],
                                start=(kb == 0), stop=last)
                            if last:
                                evict(qb)

                    if h % 2 == 1 and phases >= 4:
                        # --- normalize the finished head pair -------------
                        a = h // 2
                        recp = prc.tile([2, SEQ], f32, name="recp",
                                        tag="recp", bufs=1)
                        nc.vector.reciprocal_approx_fast(out=recp, in_=denp)
                        recb = prc.tile([P, SEQ], f32, name="recb",
                                        tag="recb")
                        # partition-broadcast each head row across its 64
                        # partitions: bounce via DRAM (linear addressing
                        # allows the 0-step partition dim; SBUF does not).
                        rd = pdram.tile([2, SEQ], f32, name="rd", tag="rd")
                        nc.sync.dma_start(out=rd, in_=recp)
                        for g in range(2):
                            nc.sync.dma_start(
                                out=recb[g * HEAD_DIM:(g + 1) * HEAD_DIM, :],
                                in_=rd[g:g + 1, :].broadcast_to(
                                    [HEAD_DIM, SEQ]))
                        nc.vector.tensor_mul(at[a], at[a], recb)

                if phases < 4:
                    dummy_out(at[0][:, 0:QB])
                    return

                # --- output projection ------------------------------------
                for mp in range(NK // 2):
                    mss = [slice((2 * mp + i) * P, (2 * mp + i + 1) * P)
                           for i in range(2)]
                    ps = [pp_mm.tile([P, QB], f32, name="pso", tag="ps")
                          for _ in range(4)]
                    for k in range(NK):
                        se = (k == 0), (k == NK - 1)
                        for i in range(2):
                            for n in range(NQB):
                                nc.tensor.matmul(
                                    ps[2 * i + n], lhsT=wo[k][:, mss[i]],
                                    rhs=at[k][:, n * QB:(n + 1) * QB],
                                    start=se[0], stop=se[1])
                    for i in range(2):
                        m = 2 * mp + i
                        for n in range(NQB):
                            ts = slice(n * QB, (n + 1) * QB)
                            ob = postg.tile([P, QB], f32, name="ob", tag="ob")
                            nc.scalar.activation(
                                out=ob, in_=ps[2 * i + n],
                                func=mybir.ActivationFunctionType.Identity,
                                bias=boe[:, m:m + 1])
                            dma(ot_d[m * P:(m + 1) * P, ts], ob)

            if reps == 1:
                body()
            else:
                with tc.For_i(0, reps, 1) as it:
                    body(it)

    nc.compile()
    return nc


def _get_module(reps=1):
    key = ("nc", reps)
    if key not in _CACHE:
        _CACHE[key] = build_module(reps)
    return _CACHE[key]


def _prep_inputs(hidden_states, Wq, bq, Wk, Wv, bv, Wo, bo):
    bf16 = ml_dtypes.bfloat16
    f32 = np.float32
    scale = f32(1.0) / f32(np.sqrt(HEAD_DIM))
    wqt = np.ascontiguousarray((Wq * scale).T).astype(bf16)
    wkt = np.ascontiguousarray(Wk.T).astype(bf16)
    wvt = np.ascontiguousarray(Wv.T).astype(bf16)
    wot = np.ascontiguousarray(Wo.T).astype(bf16)
    # biases shipped pre-permuted to [partition, e-tile] so the device DMA
    # reads contiguous lines instead of a 4-byte-strided gather.
    bqs = np.ascontiguousarray((bq * scale).reshape(NK, P).T).reshape(-1)
    bqs = bqs.astype(f32)
    boe = (bo + Wo.astype(f32) @ bv.astype(f32)).astype(f32)
    boe = np.ascontiguousarray(boe.reshape(NK, P).T).reshape(-1).astype(f32)
    shared = dict(wqt=wqt, wkt=wkt, wvt=wvt, wot=wot, bqs=bqs, boe=boe)
    in_maps = []
    for i in range(NUM_SEQS):
        xs = hidden_states[i * SEQ:(i + 1) * SEQ, :]
        xt = np.ascontiguousarray(xs.T).astype(bf16)
        in_maps.append(dict(shared, xt=xt))
    return in_maps


def _numpy_fallback(hidden_states, seq_len, Wq, bq, Wk, Wv, bv, Wo, bo):
    # Generic ragged reference (only used if seq_len deviates from 8x1024).
    T = hidden_states.shape[0]
    q = (hidden_states @ Wq.T + bq).reshape(T, HEADS, HEAD_DIM)
    k = (hidden_states @ Wk.T).reshape(T, HEADS, HEAD_DIM)
    v = (hidden_states @ Wv.T + bv).reshape(T, HEADS, HEAD_DIM)
    sl = np.asarray(seq_len).astype(np.int64)
    cu = np.concatenate([[0], np.cumsum(sl)])
    out = np.empty((T, HEADS * HEAD_DIM), np.float32)
    scale = 1.0 / np.float32(np.sqrt(HEAD_DIM))
    for b in range(len(sl)):
        s, e = int(cu[b]), int(cu[b + 1])
        qb, kb, vb = q[s:e], k[s:e], v[s:e]
        sc = np.einsum("qhd,khd->hqk", qb, kb) * scale
        L = e - s
        mask = np.tril(np.ones((L, L), bool))
        sc = np.where(mask[None], sc, -np.inf)
        sc = sc - sc.max(-1, keepdims=True)
        p = np.exp(sc)
        p /= p.sum(-1, keepdims=True)
        ob = np.einsum("hqk,khd->qhd", p, vb)
        out[s:e] = ob.reshape(L, -1)
    return (out @ Wo.T + bo).astype(np.float32)


def kernel(hidden_states, seq_len, Wq, bq, Wk, Wv, bv, Wo, bo):
    hidden_states = np.asarray(hidden_states, dtype=np.float32)
    seq_len = np.asarray(seq_len)
    Wq, bq = np.asarray(Wq, np.float32), np.asarray(bq, np.float32)
    Wk = np.asarray(Wk, np.float32)
    Wv, bv = np.asarray(Wv, np.float32), np.asarray(bv, np.float32)
    Wo, bo = np.asarray(Wo, np.float32), np.asarray(bo, np.float32)

    if (seq_len.shape != (NUM_SEQS,) or not np.all(seq_len == SEQ)
            or hidden_states.shape != (NUM_SEQS * SEQ, EMBED)):
        return _numpy_fallback(hidden_states, seq_len, Wq, bq, Wk, Wv, bv,
                               Wo, bo)

    from concourse.bass_utils import run_bass_kernel_spmd

    nc = _get_module(reps=1)
    in_maps = _prep_inputs(hidden_states, Wq, bq, Wk, Wv, bv, Wo, bo)
    res = run_bass_kernel_spmd(nc, in_maps, list(range(NUM_SEQS)))
    out = np.empty((NUM_SEQS * SEQ, EMBED), np.float32)
    for i in range(NUM_SEQS):
        out[i * SEQ:(i + 1) * SEQ, :] = res.results[i]["ot"].T
    return out



# revision 18
# speedup vs baseline: 1.1584x; 1.1584x over previous
"""Trainium2 Bass kernel for packed varlen causal attention (8 seqs x 1024 tok).

Sharding: data-parallel over sequences -- core i computes sequence i end to end.
Weights are replicated. No collectives.

Device-side math (per core, S=1024 tokens, E=1024, H=16, D=64):
  QT[e,t] = (0.125*Wq)^T-matmul, + 0.125*bq      (scale folded into weights)
  KT[e,t] = Wk^T-matmul
  V [t,e] = Wv^T-matmul, stored head-major with a ones column per head
  per head h, per q-block (512 wide):
    for k-tile (128 rows, causally live only):
      scoresT[k,q] = KT_h tile^T-matmul QT_h      (PSUM, fp32)
      p = exp(scoresT)                            (ScalarE, -> bf16 SBUF)
      causal zero-fill on the diagonal tile       (GpSimd affine_select)
      acc[d+1, q] += [V_h | 1]^T-matmul p         (PSUM accumulate)
    row d of acc = softmax denominator; rows 0..63 = unnormalized (PV)^T
  normalize with one batched reciprocal + broadcast multiply
  outT[e,t] = Wo^T-matmul A^T + (bo + Wo@bv)     (bv folded: softmax rows sum to 1)

Host glue transposes X/W (bf16) on the way in and out^T back on the way out.
"""

import numpy as np
import ml_dtypes

# Problem constants (hardcoded per the harness contract).
NUM_SEQS = 8
SEQ = 1024
EMBED = 1024
HEADS = 16
HEAD_DIM = 64
P = 128
NK = EMBED // P          # 8 contraction tiles
QB = 512                 # q-block width
NQB = SEQ // QB          # 2 q-blocks
HV = HEAD_DIM + 1        # V columns per head incl. ones column

_CACHE = {}


def build_module(reps=1):
    """Build and compile the SPMD Bass module. reps>1 wraps the body in a
    hardware loop (used only for wall-clock timing in test harnesses)."""
    import os
    import concourse.mybir as mybir
    import concourse.tile as tile
    from concourse import bacc
    from contextlib import ExitStack

    # Ablation knobs for perf bisection only; graded path uses the defaults.
    phases = int(os.environ.get("KERNEL_PHASES", "4"))
    no_evict = os.environ.get("KERNEL_NO_EVICT", "0") == "1"

    bf16 = mybir.dt.bfloat16
    f32 = mybir.dt.float32
    EXP = mybir.ActivationFunctionType.Exp

    nc = bacc.Bacc("TRN2", target_bir_lowering=False, debug=False,
                   num_devices=NUM_SEQS, num_swdge_queues=4)

    xt_d = nc.dram_tensor("xt", [EMBED, SEQ], bf16, kind="ExternalInput").ap()
    wq_d = nc.dram_tensor("wqt", [EMBED, EMBED], bf16, kind="ExternalInput").ap()
    wk_d = nc.dram_tensor("wkt", [EMBED, EMBED], bf16, kind="ExternalInput").ap()
    wv_d = nc.dram_tensor("wvt", [EMBED, EMBED], bf16, kind="ExternalInput").ap()
    wo_d = nc.dram_tensor("wot", [EMBED, EMBED], bf16, kind="ExternalInput").ap()
    bq_d = nc.dram_tensor("bqs", [EMBED], f32, kind="ExternalInput").ap()
    bo_d = nc.dram_tensor("boe", [EMBED], f32, kind="ExternalInput").ap()
    ot_d = nc.dram_tensor("ot", [EMBED, SEQ], f32, kind="ExternalOutput").ap()

    xt_v = xt_d.rearrange("(a p) t -> a p t", p=P)
    wq_v = wq_d.rearrange("(a p) e -> a p e", p=P)
    wk_v = wk_d.rearrange("(a p) e -> a p e", p=P)
    wv_v = wv_d.rearrange("(a p) e -> a p e", p=P)
    wo_v = wo_d.rearrange("(a p) e -> a p e", p=P)

    with tile.TileContext(nc) as tc:
        with ExitStack() as ctx:
            const = ctx.enter_context(tc.tile_pool(name="const", bufs=1))
            pp_mm = ctx.enter_context(
                tc.tile_pool(name="pp_mm", bufs=4, space="PSUM"))
            pp_sc = ctx.enter_context(
                tc.tile_pool(name="pp_sc", bufs=4, space="PSUM"))
            pexp = ctx.enter_context(tc.tile_pool(name="pexp", bufs=6))
            pdn = ctx.enter_context(tc.tile_pool(name="pdn", bufs=2))
            prc = ctx.enter_context(tc.tile_pool(name="prc", bufs=2))
            postg = ctx.enter_context(tc.tile_pool(name="postg", bufs=4))
            pdram = ctx.enter_context(
                tc.tile_pool(name="pdram", bufs=2, space="DRAM"))

            def body(_it=None):
                # --- persistent SBUF tensors ------------------------------
                wq = [const.tile([P, EMBED], bf16, tag=f"wq{k}", name=f"wq{k}") for k in range(NK)]
                wk = [const.tile([P, EMBED], bf16, tag=f"wk{k}", name=f"wk{k}") for k in range(NK)]
                wv = [const.tile([P, EMBED], bf16, tag=f"wv{k}", name=f"wv{k}") for k in range(NK)]
                wo = [const.tile([P, EMBED], bf16, tag=f"wo{k}", name=f"wo{k}") for k in range(NK)]
                xt = [const.tile([P, SEQ], bf16, tag=f"xt{k}", name=f"xt{k}") for k in range(NK)]
                qt = [const.tile([P, SEQ], bf16, tag=f"qt{a}", name=f"qt{a}") for a in range(NK)]
                kt = [const.tile([P, SEQ], bf16, tag=f"kt{a}", name=f"kt{a}") for a in range(NK)]
                vv = [const.tile([P, HEADS * HV], bf16, tag=f"vv{m}", name=f"vv{m}")
                      for m in range(NK)]
                at = [const.tile([P, SEQ], bf16, tag=f"at{a}", name=f"at{a}") for a in range(NK)]
                bqs = const.tile([P, NK], f32, tag="bqs")
                boe = const.tile([P, NK], f32, tag="boe")
                # explicit zero bias for Exp: a float bias would lazily
                # allocate a bass-level const tensor outside the tile pools'
                # allocator, which can land under a pool slot.
                zb = const.tile([P, 1], f32, tag="zb")
                nc.vector.memset(zb, 0.0)

                # --- loads ------------------------------------------------
                # one HWDGE queue sustains only ~22 GB/s; round-robin issue
                # over SP + ACT (HWDGE) and POOL (SWDGE) to reach ~170 GB/s.
                dma_engines = [nc.sync, nc.scalar, nc.gpsimd]
                _di = [0]

                def dma(out, in_):
                    dma_engines[_di[0] % len(dma_engines)].dma_start(
                        out=out, in_=in_)
                    _di[0] += 1

                dma(bqs, bq_d.rearrange("(p a) -> p a", a=NK))
                dma(boe, bo_d.rearrange("(p a) -> p a", a=NK))
                for k in range(NK):
                    dma(xt[k], xt_v[k])
                    dma(wv[k], wv_v[k])
                for k in range(NK):
                    dma(wq[k], wq_v[k])
                    dma(wk[k], wk_v[k])
                for k in range(NK):
                    dma(wo[k], wo_v[k])
                for m in range(NK):
                    # ones column per head for the fused denominator
                    nc.vector.memset(
                        vv[m].rearrange("p (h c) -> p h c", c=HV)[:, :, HEAD_DIM:HV],
                        1.0)

                def dummy_out(src):
                    ob = postg.tile([P, QB], f32, name="ob", tag="ob")
                    nc.vector.tensor_copy(out=ob, in_=src)
                    nc.sync.dma_start(out=ot_d[0:P, 0:QB], in_=ob)

                if phases < 2:
                    dummy_out(xt[0][:, 0:QB])
                    return

                # --- projections ------------------------------------------
                # Four interleaved PSUM accumulation chains: back-to-back
                # matmuls into the SAME bank stall the PE ~150ns each
                # (measured); round-robining 4 banks hides it, and each
                # stationary tile feeds 2 moving blocks per LDWEIGHTS.
                # V[t,e]: lhsT = X^T tile [c,t], rhs = Wv^T [c,e]
                for mp in range(NK // 2):
                    ms = [slice((2 * mp + i) * P, (2 * mp + i + 1) * P)
                          for i in range(2)]
                    ps = [pp_mm.tile([P, QB], f32, name="psv", tag="ps")
                          for _ in range(4)]
                    for k in range(NK):
                        se = (k == 0), (k == NK - 1)
                        for i in range(2):
                            for n in range(NQB):
                                nc.tensor.matmul(
                                    ps[2 * i + n], lhsT=xt[k][:, ms[i]],
                                    rhs=wv[k][:, n * QB:(n + 1) * QB],
                                    start=se[0], stop=se[1])
                    if not no_evict:
                        for i in range(2):
                            for n in range(NQB):
                                # scatter heads into the HV-strided layout
                                nc.vector.tensor_copy(
                                    out=vv[2 * mp + i]
                                    [:, n * 8 * HV:(n + 1) * 8 * HV]
                                    .rearrange("p (h c) -> p h c", c=HV)
                                    [:, :, 0:HEAD_DIM],
                                    in_=ps[2 * i + n].rearrange(
                                        "p (h c) -> p h c", c=HEAD_DIM))
                # QT[e,t], KT[e,t]: lhsT = W^T tile [c,e], rhs = X^T [c,t]
                for a in range(NK):
                    es = slice(a * P, (a + 1) * P)
                    psq = [pp_mm.tile([P, QB], f32, name="psq", tag="ps")
                           for _ in range(NQB)]
                    psk = [pp_mm.tile([P, QB], f32, name="psk", tag="ps")
                           for _ in range(NQB)]
                    for k in range(NK):
                        se = (k == 0), (k == NK - 1)
                        for n in range(NQB):
                            nc.tensor.matmul(
                                psq[n], lhsT=wq[k][:, es],
                                rhs=xt[k][:, n * QB:(n + 1) * QB],
                                start=se[0], stop=se[1])
                        for n in range(NQB):
                            nc.tensor.matmul(
                                psk[n], lhsT=wk[k][:, es],
                                rhs=xt[k][:, n * QB:(n + 1) * QB],
                                start=se[0], stop=se[1])
                    if not no_evict:
                        for n in range(NQB):
                            ts = slice(n * QB, (n + 1) * QB)
                            nc.vector.tensor_scalar(
                                out=qt[a][:, ts], in0=psq[n],
                                scalar1=bqs[:, a:a + 1], scalar2=None,
                                op0=mybir.AluOpType.add)
                            nc.vector.tensor_copy(out=kt[a][:, ts],
                                                  in_=psk[n])

                if phases < 3:
                    dummy_out(xt[0][:, 0:QB] if no_evict else qt[0][:, 0:QB])
                    return

                # --- attention --------------------------------------------
                # kb-outer / qb-inner: consecutive matmuls share stationary
                # weights (one KT tile, then one V tile), and the two q-block
                # accumulation chains interleave so PE never waits on exp.
                NKB = SEQ // P
                denp = None
                for h in range(HEADS):
                    a_h = h // 2
                    po = (h % 2) * HEAD_DIM
                    hvs = slice(h * HV, h * HV + HV)
                    acc = [pp_mm.tile([P, QB], f32, name="acc", tag="ps")
                           for qb in range(NQB)]
                    if h % 2 == 0:
                        denp = pdn.tile([2, SEQ], f32, name="denp", tag="denp")
                    dp = denp

                    def evict(qb):
                        qs = slice(qb * QB, (qb + 1) * QB)
                        # engines can only address partitions {0,32,64,96};
                        # bounce the denominator row via SBUF->SBUF DMA.
                        ds = postg.tile([1, QB], f32, name="ds", tag="ds")
                        nc.vector.tensor_copy(out=ds,
                                              in_=acc[qb][HEAD_DIM:HV, :])
                        nc.sync.dma_start(out=dp[h % 2:h % 2 + 1, qs], in_=ds)
                        nc.vector.tensor_copy(
                            out=at[a_h][po:po + HEAD_DIM, qs],
                            in_=acc[qb][0:HEAD_DIM, :])

                    for kb in range(NKB):
                        elig = [qb for qb in range(NQB)
                                if (kb + 1) * P <= (qb + 1) * QB]
                        c0 = {qb: max(0, kb * P - qb * QB) for qb in elig}
                        sc = {}
                        for qb in elig:
                            sc[qb] = pp_sc.tile([P, QB], f32, name="sc")
                            nc.tensor.matmul(
                                sc[qb][:, c0[qb]:QB],
                                lhsT=kt[a_h][po:po + HEAD_DIM,
                                             kb * P:(kb + 1) * P],
                                rhs=qt[a_h][po:po + HEAD_DIM,
                                            qb * QB + c0[qb]:(qb + 1) * QB],
                                start=True, stop=True)
                        pt = {}
                        for qb in elig:
                            pt[qb] = pexp.tile([P, QB], bf16, name="pt")
                            nc.scalar.activation(out=pt[qb][:, c0[qb]:QB],
                                                 in_=sc[qb][:, c0[qb]:QB],
                                                 func=EXP, bias=zb)
                            if kb * P >= qb * QB:
                                # diagonal tile: zero strictly-upper triangle
                                nc.gpsimd.affine_select(
                                    out=pt[qb][:, c0[qb]:c0[qb] + P],
                                    in_=pt[qb][:, c0[qb]:c0[qb] + P],
                                    compare_op=mybir.AluOpType.is_ge,
                                    fill=0.0, base=0,
                                    pattern=[[1, P]], channel_multiplier=-1)
                        for qb in elig:
                            last = kb == (qb + 1) * (QB // P) - 1
                            nc.tensor.matmul(
                                acc[qb][:HV, c0[qb]:QB], lhsT=vv[kb][:, hvs],
                                rhs=pt[qb][:, c0[qb]:QB],
                                start=(kb == 0), stop=last)
                            if last:
                                evict(qb)

                    if h % 2 == 1 and phases >= 4:
                        # --- normalize the finished head pair -------------
                        a = h // 2
                        recp = prc.tile([2, SEQ], f32, name="recp",
                                        tag="recp", bufs=1)
                        nc.vector.reciprocal_approx_fast(out=recp, in_=denp)
                        recb = prc.tile([P, SEQ], f32, name="recb",
                                        tag="recb")
                        # partition-broadcast each head row across its 64
                        # partitions: bounce via DRAM (linear addressing
                        # allows the 0-step partition dim; SBUF does not).
                        rd = pdram.tile([2, SEQ], f32, name="rd", tag="rd")
                        nc.sync.dma_start(out=rd, in_=recp)
                        for g in range(2):
                            nc.sync.dma_start(
                                out=recb[g * HEAD_DIM:(g + 1) * HEAD_DIM, :],
                                in_=rd[g:g + 1, :].broadcast_to(
                                    [HEAD_DIM, SEQ]))
                        nc.vector.tensor_mul(at[a], at[a], recb)

                if phases < 4:
                    dummy_out(at[0][:, 0:QB])
                    return

                # --- output projection ------------------------------------
                for mp in range(NK // 2):
                    mss = [slice((2 * mp + i) * P, (2 * mp + i + 1) * P)
                           for i in range(2)]
                    ps = [pp_mm.tile([P, QB], f32, name="pso", tag="ps")
                          for _ in range(4)]
                    for k in range(NK):
                        se = (k == 0), (k == NK - 1)
                        for i in range(2):
                            for n in range(NQB):
                                nc.tensor.matmul(
                                    ps[2 * i + n], lhsT=wo[k][:, mss[i]],
                                    rhs=at[k][:, n * QB:(n + 1) * QB],
                                    start=se[0], stop=se[1])
                    for i in range(2):
                        m = 2 * mp + i
                        for n in range(NQB):
                            ts = slice(n * QB, (n + 1) * QB)
                            ob = postg.tile([P, QB], f32, name="ob", tag="ob")
                            nc.scalar.activation(
                                out=ob, in_=ps[2 * i + n],
                                func=mybir.ActivationFunctionType.Identity,
                                bias=boe[:, m:m + 1])
                            dma(ot_d[m * P:(m + 1) * P, ts], ob)

            if reps == 1:
                body()
            else:
                with tc.For_i(0, reps, 1) as it:
                    body(it)

    nc.compile()
    return nc


def _get_module(reps=1):
    key = ("nc", reps)
    if key not in _CACHE:
        _CACHE[key] = build_module(reps)
    return _CACHE[key]


def _prep_inputs(hidden_states, Wq, bq, Wk, Wv, bv, Wo, bo):
    bf16 = ml_dtypes.bfloat16
    f32 = np.float32
    scale = f32(1.0) / f32(np.sqrt(HEAD_DIM))
    wqt = np.ascontiguousarray((Wq * scale).T).astype(bf16)
    wkt = np.ascontiguousarray(Wk.T).astype(bf16)
    wvt = np.ascontiguousarray(Wv.T).astype(bf16)
    wot = np.ascontiguousarray(Wo.T).astype(bf16)
    # biases shipped pre-permuted to [partition, e-tile] so the device DMA
    # reads contiguous lines instead of a 4-byte-strided gather.
    bqs = np.ascontiguousarray((bq * scale).reshape(NK, P).T).reshape(-1)
    bqs = bqs.astype(f32)
    boe = (bo + Wo.astype(f32) @ bv.astype(f32)).astype(f32)
    boe = np.ascontiguousarray(boe.reshape(NK, P).T).reshape(-1).astype(f32)
    shared = dict(wqt=wqt, wkt=wkt, wvt=wvt, wot=wot, bqs=bqs, boe=boe)
    in_maps = []
    for i in range(NUM_SEQS):
        xs = hidden_states[i * SEQ:(i + 1) * SEQ, :]
        xt = np.ascontiguousarray(xs.T).astype(bf16)
        in_maps.append(dict(shared, xt=xt))
    return in_maps


def _numpy_fallback(hidden_states, seq_len, Wq, bq, Wk, Wv, bv, Wo, bo):
    # Generic ragged reference (only used if seq_len deviates from 8x1024).
    T = hidden_states.shape[0]
    q = (hidden_states @ Wq.T + bq).reshape(T, HEADS, HEAD_DIM)
    k = (hidden_states @ Wk.T).reshape(T, HEADS, HEAD_DIM)
    v = (hidden_states @ Wv.T + bv).reshape(T, HEADS, HEAD_DIM)
    sl = np.asarray(seq_len).astype(np.int64)
    cu = np.concatenate([[0], np.cumsum(sl)])
    out = np.empty((T, HEADS * HEAD_DIM), np.float32)
    scale = 1.0 / np.float32(np.sqrt(HEAD_DIM))
    for b in range(len(sl)):
        s, e = int(cu[b]), int(cu[b + 1])
        qb, kb, vb = q[s:e], k[s:e], v[s:e]
        sc = np.einsum("qhd,khd->hqk", qb, kb) * scale
        L = e - s
        mask = np.tril(np.ones((L, L), bool))
        sc = np.where(mask[None], sc, -np.inf)
        sc = sc - sc.max(-1, keepdims=True)
        p = np.exp(sc)
        p /= p.sum(-1, keepdims=True)
        ob = np.einsum("hqk,khd->qhd", p, vb)
        out[s:e] = ob.reshape(L, -1)
    return (out @ Wo.T + bo).astype(np.float32)


def kernel(hidden_states, seq_len, Wq, bq, Wk, Wv, bv, Wo, bo):
    hidden_states = np.asarray(hidden_states, dtype=np.float32)
    seq_len = np.asarray(seq_len)
    Wq, bq = np.asarray(Wq, np.float32), np.asarray(bq, np.float32)
    Wk = np.asarray(Wk, np.float32)
    Wv, bv = np.asarray(Wv, np.float32), np.asarray(bv, np.float32)
    Wo, bo = np.asarray(Wo, np.float32), np.asarray(bo, np.float32)

    if (seq_len.shape != (NUM_SEQS,) or not np.all(seq_len == SEQ)
            or hidden_states.shape != (NUM_SEQS * SEQ, EMBED)):
        return _numpy_fallback(hidden_states, seq_len, Wq, bq, Wk, Wv, bv,
                               Wo, bo)

    from concourse.bass_utils import run_bass_kernel_spmd

    nc = _get_module(reps=1)
    in_maps = _prep_inputs(hidden_states, Wq, bq, Wk, Wv, bv, Wo, bo)
    res = run_bass_kernel_spmd(nc, in_maps, list(range(NUM_SEQS)))
    out = np.empty((NUM_SEQS * SEQ, EMBED), np.float32)
    for i in range(NUM_SEQS):
        out[i * SEQ:(i + 1) * SEQ, :] = res.results[i]["ot"].T
    return out
